# revision 9
# baseline (speedup 1.0000x reference)
"""Trainium2 Bass kernel for nn_BeliefPropagationCV (belief-propagation edge update).

Computes  y = 0.5 * ((mask * input_weight) @ input + llr_expander @ (llr_weight * llr))
for E = 4096 edges on 8 NeuronCores.

Sharding: row-shard the three [E, E] matrices (split output dim E into 8 slices of
512 rows); replicate the small vectors. Each core's shard is fed TRANSPOSED
(contraction dim j on SBUF partitions) so the TensorEngine performs the
x-weighted reduction directly via PSUM accumulation:

    y[i] = sum_j (mask.T*W.T)[j,i] * x[j] + sum_j E.T[j,i] * v[j],  v = llr_w*llr

Per 128-row j-chunk k: matmul(psum[1,512], lhsT=x[:,k:k+1], rhs=P_tile) accumulates.
The only elementwise work is one fp16 DVE multiply (mask ⊙ W) per tile.

mask / llr_expander are 0/1-valued, so the host-side fp16 cast is exact; W/x/v
are rounded to fp16 (~2^-11 relative), accumulation is fp32 in PSUM.
"""

import numpy as np

E = 4096
N_CORES = 8
R = E // N_CORES      # 512 output rows per core
P = 128               # SBUF partitions
K = E // P            # 32 contraction chunks of 128
OUTER = 8             # outer tiles (DMA/compute pipeline stages)
CPO = K // OUTER      # 4 chunks per outer tile
FREE = CPO * R        # 2048 fp16 elements per partition per outer tile


def _build_program():
    import concourse.bass as bass
    import concourse.tile as tile
    from concourse import bacc, mybir
    from contextlib import ExitStack

    f16 = mybir.dt.float16
    f32 = mybir.dt.float32

    nc = bacc.Bacc(None)
    wt = nc.dram_tensor("wt", [OUTER, P, FREE], f16, kind="ExternalInput")
    mt = nc.dram_tensor("mt", [OUTER, P, FREE], f16, kind="ExternalInput")
    et = nc.dram_tensor("et", [OUTER, P, FREE], f16, kind="ExternalInput")
    xcm = nc.dram_tensor("xcm", [P, K], f16, kind="ExternalInput")
    lcm = nc.dram_tensor("lcm", [P, K], f32, kind="ExternalInput")
    lwcm = nc.dram_tensor("lwcm", [P, K], f32, kind="ExternalInput")
    y = nc.dram_tensor("y", [R], f32, kind="ExternalOutput")

    with ExitStack() as ctx:
        tc = ctx.enter_context(tile.TileContext(nc))
        singles = ctx.enter_context(tc.tile_pool(name="singles", bufs=1))
        wp = ctx.enter_context(tc.tile_pool(name="wp", bufs=3))
        mp = ctx.enter_context(tc.tile_pool(name="mp", bufs=3))
        ep = ctx.enter_context(tc.tile_pool(name="ep", bufs=3))
        pp = ctx.enter_context(tc.tile_pool(name="pp", bufs=2))
        psp = ctx.enter_context(tc.tile_pool(name="psp", bufs=1, space="PSUM"))

        # Small replicated vectors, fed column-major ([p, k] = elem k*128+p) so
        # chunk k of the contraction dim is SBUF column k.
        xh = singles.tile([P, K], f16)
        nc.sync.dma_start(out=xh, in_=xcm[:, :])
        lf = singles.tile([P, K], f32)
        nc.sync.dma_start(out=lf, in_=lcm[:, :])
        lwf = singles.tile([P, K], f32)
        nc.sync.dma_start(out=lwf, in_=lwcm[:, :])

        vh = singles.tile([P, K], f16)
        nc.vector.tensor_mul(vh, lf, lwf)

        ps = psp.tile([1, R], f32)
        n_mm = OUTER * CPO * 2
        i_mm = 0
        for t in range(OUTER):
            w_sb = wp.tile([P, FREE], f16)
            nc.sync.dma_start(out=w_sb, in_=wt[t])
            m_sb = mp.tile([P, FREE], f16)
            nc.sync.dma_start(out=m_sb, in_=mt[t])
            e_sb = ep.tile([P, FREE], f16)
            nc.scalar.dma_start(out=e_sb, in_=et[t])
            p_sb = pp.tile([P, FREE], f16)
            nc.vector.tensor_mul(p_sb, w_sb, m_sb)
            for c in range(CPO):
                k = t * CPO + c
                sl = bass.ts(c, R)
                nc.tensor.matmul(
                    ps, xh[:, k : k + 1], p_sb[:, sl],
                    start=(i_mm == 0), stop=(i_mm == n_mm - 1),
                )
                i_mm += 1
                nc.tensor.matmul(
                    ps, vh[:, k : k + 1], e_sb[:, sl],
                    start=False, stop=(i_mm == n_mm - 1),
                )
                i_mm += 1

        # 0.5 * (term1 + term2) applied once on the tiny epilogue copy.
        ysb = singles.tile([1, R], f32)
        nc.scalar.mul(out=ysb, in_=ps, mul=0.5)
        nc.sync.dma_start(out=y[:], in_=ysb)

    # bacc passes: splits multi-waits into event semaphores (TRN2 allows at
    # most one sync wait per instruction), register allocation, etc.
    nc.compile()
    return nc


def _prep_matrix(a_rows: np.ndarray) -> np.ndarray:
    """[R, E] float -> [OUTER, P, FREE] fp16 with j on partitions.

    dram[t, p, c*R + i] = a_rows[i, (t*CPO + c)*P + p]
    """
    at = a_rows.astype(np.float16).T  # [E, R]
    return np.ascontiguousarray(
        at.reshape(OUTER, CPO, P, R).transpose(0, 2, 1, 3)
    ).reshape(OUTER, P, FREE)


def _col_major_vec(v: np.ndarray, dtype=np.float32) -> np.ndarray:
    """[E] -> [P, K] with [p, k] = v[k*P + p]."""
    return np.ascontiguousarray(v.reshape(K, P).T.astype(dtype))


def kernel(input, input_weight, mask, llr, llr_weight, llr_expander):
    from concourse.bass_utils import run_bass_kernel_spmd

    nc = _build_program()

    xcm = _col_major_vec(np.asarray(input), np.float16)
    lcm = _col_major_vec(np.asarray(llr))
    lwcm = _col_major_vec(np.asarray(llr_weight).reshape(E))

    in_maps = []
    for c in range(N_CORES):
        rows = slice(c * R, (c + 1) * R)
        in_maps.append(
            {
                "wt": _prep_matrix(np.asarray(input_weight)[rows]),
                "mt": _prep_matrix(np.asarray(mask)[rows]),
                "et": _prep_matrix(np.asarray(llr_expander)[rows]),
                "xcm": xcm,
                "lcm": lcm,
                "lwcm": lwcm,
            }
        )

    res = run_bass_kernel_spmd(nc, in_maps, core_ids=list(range(N_CORES)))
    out = np.concatenate([res.results[c]["y"] for c in range(N_CORES)])
    return out.reshape(E, 1).astype(np.float32)


# revision 15
# speedup vs baseline: 1.0900x; 1.0900x over previous
"""Trainium2 Bass kernel for nn_BeliefPropagationCV (belief-propagation edge update).

Computes  y = 0.5 * ((mask * input_weight) @ input + llr_expander @ (llr_weight * llr))
for E = 4096 edges on 8 NeuronCores.

Sharding: row-shard the three [E, E] matrices (split output dim E into 8 slices of
512 rows); replicate the small vectors. Each core's shard is fed TRANSPOSED
(contraction dim j on SBUF partitions) so the TensorEngine performs the
x-weighted reduction directly via PSUM accumulation:

    y[i] = sum_j (mask.T*W.T)[j,i] * x[j] + sum_j E.T[j,i] * v[j],  v = llr_w*llr

Per 128-row j-chunk k: matmul(psum[1,512], lhsT=x[:,k:k+1], rhs=P_tile) accumulates.
The only elementwise work is one fp16 DVE multiply (mask ⊙ W) per tile.

mask / llr_expander are 0/1-valued, so the host-side fp16 cast is exact; W/x/v
are rounded to fp16 (~2^-11 relative), accumulation is fp32 in PSUM.
"""

import numpy as np

E = 4096
N_CORES = 8
R = E // N_CORES      # 512 output rows per core
P = 128               # SBUF partitions
K = E // P            # 32 contraction chunks of 128
OUTER = 8             # outer tiles (DMA/compute pipeline stages)
CPO = K // OUTER      # 4 chunks per outer tile
FREE = CPO * R        # 2048 fp16 elements per partition per outer tile


def _build_program():
    import concourse.bass as bass
    import concourse.tile as tile
    from concourse import bacc, mybir
    from contextlib import ExitStack

    f8 = mybir.dt.float8e4
    f16 = mybir.dt.float16
    f32 = mybir.dt.float32

    nc = bacc.Bacc(None)
    wt = nc.dram_tensor("wt", [OUTER, P, FREE], f16, kind="ExternalInput")
    # mask / llr_expander are 0/1-valued: fp8_e4m3 is exact and halves traffic.
    mt = nc.dram_tensor("mt", [OUTER, P, FREE], f8, kind="ExternalInput")
    et = nc.dram_tensor("et", [OUTER, P, FREE], f8, kind="ExternalInput")
    xcm = nc.dram_tensor("xcm", [P, K], f16, kind="ExternalInput")
    lvw = nc.dram_tensor("lvw", [P, 2 * K], f32, kind="ExternalInput")
    y = nc.dram_tensor("y", [R], f32, kind="ExternalOutput")

    with ExitStack() as ctx:
        tc = ctx.enter_context(tile.TileContext(nc))
        singles = ctx.enter_context(tc.tile_pool(name="singles", bufs=1))
        wp = ctx.enter_context(tc.tile_pool(name="wp", bufs=4))
        mp = ctx.enter_context(tc.tile_pool(name="mp", bufs=4))
        ep = ctx.enter_context(tc.tile_pool(name="ep", bufs=4))
        pp = ctx.enter_context(tc.tile_pool(name="pp", bufs=3))
        psp = ctx.enter_context(tc.tile_pool(name="psp", bufs=1, space="PSUM"))

        # Small replicated vectors, fed column-major ([p, k] = elem k*128+p) so
        # chunk k of the contraction dim is SBUF column k.
        xh = singles.tile([P, K], f16)
        nc.sync.dma_start(out=xh, in_=xcm[:, :])
        lvf = singles.tile([P, 2 * K], f32)
        nc.sync.dma_start(out=lvf, in_=lvw[:, :])

        vh = singles.tile([P, K], f16)
        nc.vector.tensor_mul(vh, lvf[:, :K], lvf[:, K:])

        ps = psp.tile([1, R], f32)
        n_mm = OUTER * CPO * 2
        i_mm = 0
        for t in range(OUTER):
            w_sb = wp.tile([P, FREE], f16)
            nc.sync.dma_start(out=w_sb, in_=wt[t])
            # SWDGE casts fp8 -> fp16 in the DMA datapath (HBM reads 1B/elem).
            m_sb = mp.tile([P, FREE], f16)
            nc.gpsimd.dma_start(out=m_sb, in_=mt[t])
            e_sb = ep.tile([P, FREE], f16)
            nc.gpsimd.dma_start(out=e_sb, in_=et[t])
            p_sb = pp.tile([P, FREE], f16)
            nc.vector.tensor_mul(p_sb, w_sb, m_sb)
            for c in range(CPO):
                k = t * CPO + c
                sl = bass.ts(c, R)
                nc.tensor.matmul(
                    ps, xh[:, k : k + 1], p_sb[:, sl],
                    start=(i_mm == 0), stop=(i_mm == n_mm - 1),
                )
                i_mm += 1
                nc.tensor.matmul(
                    ps, vh[:, k : k + 1], e_sb[:, sl],
                    start=False, stop=(i_mm == n_mm - 1),
                )
                i_mm += 1

        # 0.5 * (term1 + term2) applied once on the tiny epilogue copy.
        ysb = singles.tile([1, R], f32)
        nc.scalar.mul(out=ysb, in_=ps, mul=0.5)
        nc.sync.dma_start(out=y[:], in_=ysb)

    # bacc passes: splits multi-waits into event semaphores (TRN2 allows at
    # most one sync wait per instruction), register allocation, etc.
    nc.compile()
    return nc


def _prep_matrix(a_rows: np.ndarray, dtype=np.float16) -> np.ndarray:
    """[R, E] float -> [OUTER, P, FREE] with j on partitions.

    dram[t, p, c*R + i] = a_rows[i, (t*CPO + c)*P + p]
    """
    at = a_rows.astype(dtype).T  # [E, R]
    return np.ascontiguousarray(
        at.reshape(OUTER, CPO, P, R).transpose(0, 2, 1, 3)
    ).reshape(OUTER, P, FREE)


def _f8_dtype():
    from concourse import mybir

    return mybir.dt.np(mybir.dt.float8e4)


def _col_major_vec(v: np.ndarray, dtype=np.float32) -> np.ndarray:
    """[E] -> [P, K] with [p, k] = v[k*P + p]."""
    return np.ascontiguousarray(v.reshape(K, P).T.astype(dtype))


def _make_in_maps(input, input_weight, mask, llr, llr_weight, llr_expander):
    f8 = _f8_dtype()
    xcm = _col_major_vec(np.asarray(input), np.float16)
    lvw = np.concatenate(
        [
            _col_major_vec(np.asarray(llr)),
            _col_major_vec(np.asarray(llr_weight).reshape(E)),
        ],
        axis=1,
    )

    in_maps = []
    for c in range(N_CORES):
        rows = slice(c * R, (c + 1) * R)
        in_maps.append(
            {
                "wt": _prep_matrix(np.asarray(input_weight)[rows]),
                "mt": _prep_matrix(np.asarray(mask)[rows], f8),
                "et": _prep_matrix(np.asarray(llr_expander)[rows], f8),
                "xcm": xcm,
                "lvw": lvw,
            }
        )
    return in_maps


def kernel(input, input_weight, mask, llr, llr_weight, llr_expander):
    from concourse.bass_utils import run_bass_kernel_spmd

    nc = _build_program()
    in_maps = _make_in_maps(input, input_weight, mask, llr, llr_weight, llr_expander)
    res = run_bass_kernel_spmd(nc, in_maps, core_ids=list(range(N_CORES)))
    out = np.concatenate([res.results[c]["y"] for c in range(N_CORES)])
    return out.reshape(E, 1).astype(np.float32)


# revision 17
# speedup vs baseline: 1.2592x; 1.1552x over previous
"""Trainium2 Bass kernel for nn_BeliefPropagationCV (belief-propagation edge update).

Computes  y = 0.5 * ((mask * input_weight) @ input + llr_expander @ (llr_weight * llr))
for E = 4096 edges on 8 NeuronCores.

Sharding: row-shard the three [E, E] matrices (split output dim E into 8 slices of
512 rows); replicate the small vectors. Each core's shard is fed TRANSPOSED
(contraction dim j on SBUF partitions) so the TensorEngine performs the
x-weighted reduction directly via PSUM accumulation:

    y[i] = sum_j (mask.T*W.T)[j,i] * x[j] + sum_j E.T[j,i] * v[j],  v = llr_w*llr

Per 128-row j-chunk k: matmul(psum[1,512], lhsT=x[:,k:k+1], rhs=P_tile) accumulates.
The only elementwise work is one fp16 DVE multiply (mask ⊙ W) per tile.

mask / llr_expander are 0/1-valued, so the host-side fp16 cast is exact; W/x/v
are rounded to fp16 (~2^-11 relative), accumulation is fp32 in PSUM.
"""

import numpy as np

E = 4096
N_CORES = 8
R = E // N_CORES      # 512 output rows per core
P = 128               # SBUF partitions
K = E // P            # 32 contraction chunks of 128
OUTER = 8             # outer tiles (DMA/compute pipeline stages)
CPO = K // OUTER      # 4 chunks per outer tile
FREE = CPO * R        # 2048 fp16 elements per partition per outer tile


def _build_program():
    import concourse.bass as bass
    import concourse.tile as tile
    from concourse import bacc, mybir
    from contextlib import ExitStack

    f8 = mybir.dt.float8e4
    f16 = mybir.dt.float16
    f32 = mybir.dt.float32

    nc = bacc.Bacc(None)
    wt = nc.dram_tensor("wt", [OUTER, P, FREE], f16, kind="ExternalInput")
    # mask / llr_expander are 0/1-valued: fp8_e4m3 is exact and halves traffic.
    mt = nc.dram_tensor("mt", [OUTER, P, FREE], f8, kind="ExternalInput")
    et = nc.dram_tensor("et", [OUTER, P, FREE], f8, kind="ExternalInput")
    xcm = nc.dram_tensor("xcm", [P, K], f16, kind="ExternalInput")
    lvw = nc.dram_tensor("lvw", [P, 2 * K], f32, kind="ExternalInput")
    y = nc.dram_tensor("y", [R], f32, kind="ExternalOutput")

    with ExitStack() as ctx:
        tc = ctx.enter_context(tile.TileContext(nc))
        singles = ctx.enter_context(tc.tile_pool(name="singles", bufs=1))
        wp = ctx.enter_context(tc.tile_pool(name="wp", bufs=6))
        mp = ctx.enter_context(tc.tile_pool(name="mp", bufs=6))
        ep = ctx.enter_context(tc.tile_pool(name="ep", bufs=6))
        pp = ctx.enter_context(tc.tile_pool(name="pp", bufs=4))
        psp = ctx.enter_context(tc.tile_pool(name="psp", bufs=1, space="PSUM"))

        # Small replicated vectors, fed column-major ([p, k] = elem k*128+p) so
        # chunk k of the contraction dim is SBUF column k.
        xh = singles.tile([P, K], f16)
        nc.sync.dma_start(out=xh, in_=xcm[:, :])
        lvf = singles.tile([P, 2 * K], f32)
        nc.sync.dma_start(out=lvf, in_=lvw[:, :])

        vh = singles.tile([P, K], f16)
        nc.vector.tensor_mul(vh, lvf[:, :K], lvf[:, K:])

        ps = psp.tile([1, R], f32)
        n_mm = OUTER * CPO * 2
        i_mm = 0
        for t in range(OUTER):
            # Split the two HWDGE rings: W on SP, mask+expander on ACT.
            w_sb = wp.tile([P, FREE], f16)
            nc.sync.dma_start(out=w_sb, in_=wt[t])
            m_sb = mp.tile([P, FREE], f8)
            nc.scalar.dma_start(out=m_sb, in_=mt[t])
            e_sb = ep.tile([P, FREE], f8)
            nc.scalar.dma_start(out=e_sb, in_=et[t])
            # Mixed-dtype multiply: fp16 W x fp8 mask -> fp16 product.
            p_sb = pp.tile([P, FREE], f16)
            nc.vector.tensor_mul(p_sb, w_sb, m_sb)
            for c in range(CPO):
                k = t * CPO + c
                sl = bass.ts(c, R)
                nc.tensor.matmul(
                    ps, xh[:, k : k + 1], p_sb[:, sl],
                    start=(i_mm == 0), stop=(i_mm == n_mm - 1),
                )
                i_mm += 1
                nc.tensor.matmul(
                    ps, vh[:, k : k + 1], e_sb[:, sl],
                    start=False, stop=(i_mm == n_mm - 1),
                )
                i_mm += 1

        # 0.5 * (term1 + term2) applied once on the tiny epilogue copy.
        ysb = singles.tile([1, R], f32)
        nc.scalar.mul(out=ysb, in_=ps, mul=0.5)
        nc.sync.dma_start(out=y[:], in_=ysb)

    # bacc passes: splits multi-waits into event semaphores (TRN2 allows at
    # most one sync wait per instruction), register allocation, etc.
    nc.compile()
    return nc


def _prep_matrix(a_rows: np.ndarray, dtype=np.float16) -> np.ndarray:
    """[R, E] float -> [OUTER, P, FREE] with j on partitions.

    dram[t, p, c*R + i] = a_rows[i, (t*CPO + c)*P + p]
    """
    at = a_rows.astype(dtype).T  # [E, R]
    return np.ascontiguousarray(
        at.reshape(OUTER, CPO, P, R).transpose(0, 2, 1, 3)
    ).reshape(OUTER, P, FREE)


def _f8_dtype():
    from concourse import mybir

    return mybir.dt.np(mybir.dt.float8e4)


def _col_major_vec(v: np.ndarray, dtype=np.float32) -> np.ndarray:
    """[E] -> [P, K] with [p, k] = v[k*P + p]."""
    return np.ascontiguousarray(v.reshape(K, P).T.astype(dtype))


def _make_in_maps(input, input_weight, mask, llr, llr_weight, llr_expander):
    f8 = _f8_dtype()
    xcm = _col_major_vec(np.asarray(input), np.float16)
    lvw = np.concatenate(
        [
            _col_major_vec(np.asarray(llr)),
            _col_major_vec(np.asarray(llr_weight).reshape(E)),
        ],
        axis=1,
    )

    in_maps = []
    for c in range(N_CORES):
        rows = slice(c * R, (c + 1) * R)
        in_maps.append(
            {
                "wt": _prep_matrix(np.asarray(input_weight)[rows]),
                "mt": _prep_matrix(np.asarray(mask)[rows], f8),
                "et": _prep_matrix(np.asarray(llr_expander)[rows], f8),
                "xcm": xcm,
                "lvw": lvw,
            }
        )
    return in_maps


def kernel(input, input_weight, mask, llr, llr_weight, llr_expander):
    from concourse.bass_utils import run_bass_kernel_spmd

    nc = _build_program()
    in_maps = _make_in_maps(input, input_weight, mask, llr, llr_weight, llr_expander)
    res = run_bass_kernel_spmd(nc, in_maps, core_ids=list(range(N_CORES)))
    out = np.concatenate([res.results[c]["y"] for c in range(N_CORES)])
    return out.reshape(E, 1).astype(np.float32)
